# revision 5
# baseline (speedup 1.0000x reference)
"""AverageSpanExtractor Trainium2 kernel (v3: hardware indirect-DMA gather).

Math: out[b, n, :] = mean(seq[b, s_n:e_n, :]) * mask[b, n]

Per core (data-parallel over batch across 8 cores):
  1. Load seq [S=2048, D=512] f32, cast fp16.
  2. Per 128-token block: block-diag strict-upper matmul gives the 32-row
     SUB-BLOCK-local exclusive prefix L[r] = sum(seq[32*(r>>5) .. r)); cast
     fp16 and store to a DRAM table [2048, 512] fp16.  |L| stays < ~40 so
     fp16 rounding error is ~1e-2 absolute worst-case, well inside the gate.
  3. Gather tbl[e_n] and tbl[s_n] for all spans with indirect hardware DMAs
     (qPoolDynamic; per-partition indices are the RAW span starts/ends since
     the table is exclusive -- no -1 shift, no masking, no gpsimd ucode).
  4. The missing inter-sub-block offsets C[b] = sum of sub-block totals
     [0, b), b in [0, 64), are added exactly via a +-onehot fp16 matmul
     (onehot over e>>5 / s>>5 built from threshold compares) against
     hi/lo-split C vectors.
  5. Per span tile, PSUM accumulates onehot@C + Id@tbl[e] - Id@tbl[s]; one
     ACT pass scales by mask/width and the result is stored.
"""

import numpy as np

import concourse.bacc as bacc
import concourse.bass as bass
import concourse.tile as tile
from concourse import mybir
from concourse.bass import AP, IndirectOffsetOnAxis

# Problem shape (hardcoded per contract).
B, S, D, N = 8, 2048, 512, 1024
NBLK = S // 128          # 16 token blocks (matmul granularity)
SUB = 32                 # sub-block rows (fp16 table locality)
NSUB = S // SUB          # 64 sub-blocks (offset-table entries)
SPB = 128 // SUB         # 4 sub-blocks per 128-block
NTILE = N // 128         # 8 span tiles

F32 = mybir.dt.float32
F16 = mybir.dt.float16
I32 = mybir.dt.int32

GPAIR = 2                # span tiles gathered per indirect DMA


def build_kernel_body(tc: tile.TileContext, seq: AP, spans: AP, maskw: AP,
                      out: AP, tbl, consts, ctx, dbg=None):
    nc = tc.nc
    const = ctx.enter_context(tc.tile_pool(name="const", bufs=1))
    sbuf = ctx.enter_context(tc.tile_pool(name="sbuf", bufs=1))
    opool = ctx.enter_context(tc.tile_pool(name="opool", bufs=3))
    psum_b = ctx.enter_context(tc.tile_pool(name="pb", bufs=2, space="PSUM"))
    psum_off = ctx.enter_context(tc.tile_pool(name="poff", bufs=1, space="PSUM"))
    psum_c = ctx.enter_context(tc.tile_pool(name="pc", bufs=4, space="PSUM"))

    # ---------------- constants (NEFF Const DRAM -> SBUF, sync queue) -------
    u_strict = const.tile([128, 128], F16, tag="u_strict")
    nc.sync.dma_start(u_strict[:], consts["u_strict"].ap())
    idp = const.tile([128, 128], F16, tag="idp")
    nc.sync.dma_start(idp[:], consts["idp"].ap())
    idn = const.tile([128, 128], F16, tag="idn")
    nc.sync.dma_start(idn[:], consts["idn"].ap())
    u64s = const.tile([NSUB, NSUB], F16, tag="u64s")
    nc.sync.dma_start(u64s[:], consts["u64s"].ap())
    thr_lo = const.tile([NSUB, 1], F32, tag="thr_lo")
    nc.sync.dma_start(thr_lo[:], consts["thr_lo"].ap())
    thr_hi = const.tile([NSUB, 1], F32, tag="thr_hi")
    nc.sync.dma_start(thr_hi[:], consts["thr_hi"].ap())

    # ---------------- span staging ----------------------------------------
    # span-major [p, j]: span n = 128j + p.  These int32 tiles are ALSO the
    # indirect-gather index tables (exclusive table => raw s, e indices).
    st_pj = sbuf.tile([128, NTILE], I32, tag="st_pj")
    en_pj = sbuf.tile([128, NTILE], I32, tag="en_pj")
    mk_pj = sbuf.tile([128, NTILE], I32, tag="mk_pj")
    nc.sync.dma_start(st_pj[:], AP(spans.tensor, 0, [[2, 128], [256, NTILE]]))
    nc.sync.dma_start(en_pj[:], AP(spans.tensor, 1, [[2, 128], [256, NTILE]]))
    nc.sync.dma_start(mk_pj[:], AP(maskw.tensor, 0, [[1, 128], [128, NTILE]]))

    # free-major onehot rows: cast (e|s) to fp16 (exact, values < 2048), PE
    # transpose [128, 16] -> [16, 128], fold partitions into [1, 1024] rows
    # (span n = 128j + p), replicate to 64 partitions.
    es16 = sbuf.tile([128, 2 * NTILE], F16, tag="es16")
    nc.vector.tensor_copy(es16[:, 0:NTILE], en_pj[:])
    nc.vector.tensor_copy(es16[:, NTILE:2 * NTILE], st_pj[:])
    psT = psum_off.tile([2 * NTILE, 128], F16, tag="psT")
    nc.tensor.transpose(out=psT[:], in_=es16[:], identity=idp[:])
    esT = sbuf.tile([2 * NTILE, 128], F32, tag="esT")
    nc.vector.tensor_copy(esT[:], psT[:])

    ef = sbuf.tile([NSUB, N], F32, tag="ef")
    sf = sbuf.tile([NSUB, N], F32, tag="sf")
    nc.scalar.dma_start(ef[0:1, :].rearrange("p (j c) -> p j c", j=NTILE),
                        esT[0:NTILE, :])
    nc.scalar.dma_start(sf[0:1, :].rearrange("p (j c) -> p j c", j=NTILE),
                        esT[NTILE:2 * NTILE, :])
    for k in (1, 2, 4, 8, 16, 32):
        nc.scalar.dma_start(ef[k:2 * k, :], ef[0:k, :])
        nc.scalar.dma_start(sf[k:2 * k, :], sf[0:k, :])

    # onehot[k, n] = [sub(e_n) == k] - [sub(s_n) == k], via threshold compares
    cme = sbuf.tile([NSUB, N], F32, tag="cme")
    nc.vector.tensor_scalar(out=cme[:], in0=ef[:], scalar1=thr_lo[:],
                            scalar2=None, op0=mybir.AluOpType.is_ge)
    tmp = sbuf.tile([NSUB, N], F32, tag="tmp")
    nc.vector.tensor_scalar(out=tmp[:], in0=ef[:], scalar1=thr_hi[:],
                            scalar2=None, op0=mybir.AluOpType.is_ge)
    nc.vector.tensor_tensor(out=cme[:], in0=cme[:], in1=tmp[:],
                            op=mybir.AluOpType.subtract)
    cms = sbuf.tile([NSUB, N], F32, tag="cms")
    nc.vector.tensor_scalar(out=cms[:], in0=sf[:], scalar1=thr_lo[:],
                            scalar2=None, op0=mybir.AluOpType.is_ge)
    nc.vector.tensor_scalar(out=tmp[:], in0=sf[:], scalar1=thr_hi[:],
                            scalar2=None, op0=mybir.AluOpType.is_ge)
    nc.vector.tensor_tensor(out=cms[:], in0=cms[:], in1=tmp[:],
                            op=mybir.AluOpType.subtract)
    # oh rows 0:64 select C_hi, rows 64:128 the same pattern for C_lo
    oh = sbuf.tile([128, N], F16, tag="oh")
    nc.vector.tensor_tensor(out=oh[0:NSUB, :], in0=cme[:], in1=cms[:],
                            op=mybir.AluOpType.subtract)
    nc.scalar.dma_start(oh[NSUB:128, :], oh[0:NSUB, :])

    # per-span scale = mask / width
    w_i = sbuf.tile([128, NTILE], I32, tag="w_i")
    nc.vector.tensor_tensor(out=w_i[:], in0=en_pj[:], in1=st_pj[:],
                            op=mybir.AluOpType.subtract)
    w_f = sbuf.tile([128, NTILE], F32, tag="w_f")
    nc.vector.tensor_copy(w_f[:], w_i[:])
    r_f = sbuf.tile([128, NTILE], F32, tag="r_f")
    nc.vector.reciprocal(r_f[:], w_f[:])
    m_f = sbuf.tile([128, NTILE], F32, tag="m_f")
    nc.vector.tensor_copy(m_f[:], mk_pj[:])
    scale = sbuf.tile([128, NTILE], F32, tag="scale")
    nc.vector.tensor_tensor(out=scale[:], in0=r_f[:], in1=m_f[:],
                            op=mybir.AluOpType.mult)

    # ---------------- seq load + fp16 cast + local cumsum + table store ----
    xbig = sbuf.tile([128, NBLK, D], F32, tag="xbig")
    xf = sbuf.tile([128, NBLK, D], F16, tag="xf")
    tbl_sb = sbuf.tile([128, NBLK, D], F16, tag="tbl_sb")
    for q in range(NBLK // 4):
        sl = (slice(None), slice(4 * q, 4 * q + 4), slice(None))
        nc.sync.dma_start(
            xbig[sl],
            seq[512 * q:512 * (q + 1), :].rearrange("(j p) d -> p j d", p=128))
        nc.vector.tensor_copy(xf[sl], xbig[sl])
        for b in range(4 * q, 4 * q + 4):
            pl = psum_b.tile([128, D], F32, tag="pb")
            nc.tensor.matmul(out=pl[:], lhsT=u_strict[:], rhs=xf[:, b, :],
                             start=True, stop=True)
            if b % 2 == 0:
                nc.scalar.copy(tbl_sb[:, b, :], pl[:])
            else:
                nc.vector.tensor_copy(tbl_sb[:, b, :], pl[:])
        # store 4 blocks (512 rows) to the DRAM table, scalar queue
        nc.scalar.dma_start(
            AP(tbl, 512 * D * q, [[D, 128], [128 * D, 4], [1, D]]),
            tbl_sb[:, 4 * q:4 * q + 4, :])

    # ---------------- sub-block totals -> offsets C ------------------------
    # T_b = L[32b + 31] + x[32b + 31]; stored in PERMUTED row order
    # k' = 16i + blk (i = sub-in-block, blk = 128-block); the u64s const's
    # rows are permuted to match, its columns are in true order b.
    t16f = sbuf.tile([NSUB, D], F16, tag="t16f")
    xrow = sbuf.tile([NSUB, D], F16, tag="xrow")
    for i in range(SPB):
        p = SUB * i + SUB - 1
        nc.sync.dma_start(t16f[16 * i:16 * (i + 1), :], tbl_sb[p:p + 1, :, :])
        nc.sync.dma_start(xrow[16 * i:16 * (i + 1), :], xf[p:p + 1, :, :])
    t16 = sbuf.tile([NSUB, D], F32, tag="t16")
    nc.vector.tensor_tensor(out=t16[:], in0=t16f[:], in1=xrow[:],
                            op=mybir.AluOpType.add)
    # hi/lo fp16 split of T, permuted strict-upper matmul -> C = cf (f32)
    th = sbuf.tile([NSUB, D], F16, tag="th")
    nc.vector.tensor_copy(th[:], t16[:])
    tl = sbuf.tile([NSUB, D], F16, tag="tl")
    nc.vector.tensor_tensor(out=tl[:], in0=t16[:], in1=th[:],
                            op=mybir.AluOpType.subtract)
    poff = psum_off.tile([NSUB, D], F32, tag="poff")
    nc.tensor.matmul(out=poff[:], lhsT=u64s[:], rhs=th[:], start=True, stop=False)
    nc.tensor.matmul(out=poff[:], lhsT=u64s[:], rhs=tl[:], start=False, stop=True)
    cf = sbuf.tile([NSUB, D], F32, tag="cf")
    nc.vector.tensor_copy(cf[:], poff[:])
    # hi/lo fp16 split of C, packed [128, 512] for the onehot matmul
    chi = sbuf.tile([NSUB, D], F16, tag="chi")
    nc.vector.tensor_copy(chi[:], cf[:])
    clo = sbuf.tile([NSUB, D], F16, tag="clo")
    nc.vector.tensor_tensor(out=clo[:], in0=cf[:], in1=chi[:],
                            op=mybir.AluOpType.subtract)
    chiclo = sbuf.tile([128, D], F16, tag="chiclo")
    nc.scalar.dma_start(chiclo[0:NSUB, :], chi[:])
    nc.scalar.dma_start(chiclo[NSUB:128, :], clo[:])

    # ---------------- indirect gathers (qPoolDynamic) ---------------------
    ge_all = sbuf.tile([128, NTILE, D], F16, tag="ge_all")
    gs_all = sbuf.tile([128, NTILE, D], F16, tag="gs_all")
    tbl_ap = AP(tbl, 0, [[D, S], [1, D]])
    for g in range(NTILE // GPAIR):
        jsl = slice(GPAIR * g, GPAIR * (g + 1))
        nc.gpsimd.indirect_dma_start(
            out=ge_all[:, jsl, :], out_offset=None, in_=tbl_ap,
            in_offset=IndirectOffsetOnAxis(ap=en_pj[:, jsl], axis=0))
        nc.gpsimd.indirect_dma_start(
            out=gs_all[:, jsl, :], out_offset=None, in_=tbl_ap,
            in_offset=IndirectOffsetOnAxis(ap=st_pj[:, jsl], axis=0))

    # ---------------- combine: PSUM = onehot@C + tbl[e] - tbl[s] ----------
    for j in range(NTILE):
        ps = psum_c.tile([128, D], F32, tag="pc")
        nc.tensor.matmul(out=ps[:], lhsT=oh[:, 128 * j:128 * (j + 1)],
                         rhs=chiclo[:], start=True, stop=False)
        nc.tensor.matmul(out=ps[:], lhsT=idp[:], rhs=ge_all[:, j, :],
                         start=False, stop=False)
        nc.tensor.matmul(out=ps[:], lhsT=idn[:], rhs=gs_all[:, j, :],
                         start=False, stop=True)
        o_t = opool.tile([128, D], F32, tag="o")
        nc.scalar.mul(o_t[:], ps[:], scale[:, j:j + 1])
        nc.sync.dma_start(out[128 * j:128 * (j + 1), :], o_t[:])

    if dbg is not None:
        nc.sync.dma_start(dbg["tbl_sb"][:], tbl_sb[:])
        nc.sync.dma_start(dbg["ge"][:], ge_all[:])
        nc.sync.dma_start(dbg["gs"][:], gs_all[:])
        nc.sync.dma_start(dbg["oh"][:], oh[:])
        nc.sync.dma_start(dbg["cf"][:], cf[:])
        nc.sync.dma_start(dbg["scale"][:], scale[:])
        nc.sync.dma_start(dbg["t16"][:], t16[:])


def _make_consts(nc):
    # strict-upper within each 32-row sub-block, block-diagonal
    r = np.arange(128)
    ustrict = ((r[:, None] < r[None, :]) &
               (r[:, None] // SUB == r[None, :] // SUB)).astype(np.float16)
    # u64s[k', b] = [true_b(k') < b] with k' = 16i + blk -> true_b = 4*blk + i
    kp = np.arange(NSUB)
    true_b = SPB * (kp % 16) + kp // 16
    u64s = (true_b[:, None] < np.arange(NSUB)[None, :]).astype(np.float16)
    idp = np.eye(128, dtype=np.float16)
    idn = -np.eye(128, dtype=np.float16)
    k = np.arange(NSUB)
    thr_lo = (float(SUB) * k).astype(np.float32).reshape(NSUB, 1)
    thr_hi = (float(SUB) * (k + 1)).astype(np.float32).reshape(NSUB, 1)
    return {
        "u_strict": nc.inline_tensor(ustrict, name="c_ustrict"),
        "u64s": nc.inline_tensor(u64s, name="c_u64s"),
        "idp": nc.inline_tensor(idp, name="c_idp"),
        "idn": nc.inline_tensor(idn, name="c_idn"),
        "thr_lo": nc.inline_tensor(thr_lo, name="c_thrlo"),
        "thr_hi": nc.inline_tensor(thr_hi, name="c_thrhi"),
    }


def build_nc(debug_taps=False):
    nc = bacc.Bacc("TRN2", target_bir_lowering=False, debug=False)
    seq = nc.dram_tensor("seq", [S, D], F32, kind="ExternalInput")
    spans = nc.dram_tensor("spans", [N, 2], I32, kind="ExternalInput")
    maskw = nc.dram_tensor("maskw", [N], I32, kind="ExternalInput")
    out = nc.dram_tensor("out", [N, D], F32, kind="ExternalOutput")
    tbl = nc.dram_tensor("tbl", [S, D], F16, kind="Internal")
    consts = _make_consts(nc)
    dbg = None
    if debug_taps:
        dbg = {
            "tbl_sb": nc.dram_tensor("dbg_tbl", [128, NBLK, D], F16,
                                     kind="ExternalOutput").ap(),
            "ge": nc.dram_tensor("dbg_ge", [128, NTILE, D], F16,
                                 kind="ExternalOutput").ap(),
            "gs": nc.dram_tensor("dbg_gs", [128, NTILE, D], F16,
                                 kind="ExternalOutput").ap(),
            "oh": nc.dram_tensor("dbg_oh", [128, N], F16,
                                 kind="ExternalOutput").ap(),
            "cf": nc.dram_tensor("dbg_cf", [NSUB, D], F32,
                                 kind="ExternalOutput").ap(),
            "scale": nc.dram_tensor("dbg_scale", [128, NTILE], F32,
                                    kind="ExternalOutput").ap(),
            "t16": nc.dram_tensor("dbg_t16", [NSUB, D], F32,
                                  kind="ExternalOutput").ap(),
        }
    from contextlib import ExitStack
    with tile.TileContext(nc) as tc:
        with ExitStack() as ctx:
            build_kernel_body(tc, seq.ap(), spans.ap(), maskw.ap(), out.ap(),
                              tbl, consts, ctx, dbg=dbg)
    nc.compile()
    return nc


_NC_CACHE = None


def kernel(sequence_tensor: np.ndarray, span_indices: np.ndarray,
           span_indices_mask: np.ndarray) -> np.ndarray:
    global _NC_CACHE
    from concourse.bass_utils import run_bass_kernel_spmd

    if _NC_CACHE is None:
        _NC_CACHE = build_nc()
    nc = _NC_CACHE

    spans_i32 = np.ascontiguousarray(np.asarray(span_indices).astype(np.int32))
    mask_i32 = np.ascontiguousarray(np.asarray(span_indices_mask).astype(np.int32))
    seq_f32 = np.ascontiguousarray(sequence_tensor, dtype=np.float32)

    in_maps = [
        {"seq": seq_f32[b], "spans": spans_i32[b], "maskw": mask_i32[b]}
        for b in range(B)
    ]
    res = run_bass_kernel_spmd(nc, in_maps, core_ids=list(range(B)))
    return np.stack([r["out"] for r in res.results], axis=0)


# revision 8
# speedup vs baseline: 5.0631x; 5.0631x over previous
"""AverageSpanExtractor Trainium2 kernel (v3: hardware indirect-DMA gather).

Math: out[b, n, :] = mean(seq[b, s_n:e_n, :]) * mask[b, n]

Per core (data-parallel over batch across 8 cores):
  1. Load seq [S=2048, D=512] f32, cast fp16.
  2. Per 128-token block: block-diag strict-upper matmul gives the 32-row
     SUB-BLOCK-local exclusive prefix L[r] = sum(seq[32*(r>>5) .. r)); cast
     fp16 and store to a DRAM table [2048, 512] fp16.  |L| stays < ~40 so
     fp16 rounding error is ~1e-2 absolute worst-case, well inside the gate.
  3. Gather tbl[e_n] and tbl[s_n] for all spans with indirect hardware DMAs
     (qPoolDynamic; per-partition indices are the RAW span starts/ends since
     the table is exclusive -- no -1 shift, no masking, no gpsimd ucode).
  4. The missing inter-sub-block offsets C[b] = sum of sub-block totals
     [0, b), b in [0, 64), are added exactly via a +-onehot fp16 matmul
     (onehot over e>>5 / s>>5 built from threshold compares) against
     hi/lo-split C vectors.
  5. Per span tile, PSUM accumulates onehot@C + Id@tbl[e] - Id@tbl[s]; one
     ACT pass scales by mask/width and the result is stored.
"""

import numpy as np

import concourse.bacc as bacc
import concourse.bass as bass
import concourse.tile as tile
from concourse import mybir
from concourse.bass import AP
from concourse.library_config import mlp
from concourse.tile_rust import add_dep_helper

# Problem shape (hardcoded per contract).
B, S, D, N = 8, 2048, 512, 1024
NBLK = S // 128          # 16 token blocks (matmul granularity)
SUB = 32                 # sub-block rows (fp16 table locality)
NSUB = S // SUB          # 64 sub-blocks (offset-table entries)
SPB = 128 // SUB         # 4 sub-blocks per 128-block
NTILE = N // 128         # 8 span tiles

F32 = mybir.dt.float32
F16 = mybir.dt.float16
I32 = mybir.dt.int32
I16 = mybir.dt.int16

NGATHER = 4              # gather instructions (2 span tiles each)


def build_kernel_body(tc: tile.TileContext, seq: AP, spans: AP, maskw: AP,
                      out: AP, tbl, consts, ctx, dbg=None):
    nc = tc.nc
    const = ctx.enter_context(tc.tile_pool(name="const", bufs=1))
    sbuf = ctx.enter_context(tc.tile_pool(name="sbuf", bufs=1))
    opool = ctx.enter_context(tc.tile_pool(name="opool", bufs=3))
    gpool = ctx.enter_context(tc.tile_pool(name="gpool", bufs=1))
    psum_b = ctx.enter_context(tc.tile_pool(name="pb", bufs=2, space="PSUM"))
    psum_off = ctx.enter_context(tc.tile_pool(name="poff", bufs=1, space="PSUM"))
    psum_c = ctx.enter_context(tc.tile_pool(name="pc", bufs=4, space="PSUM"))

    # library load first: ~20-25us on the Pool engine, overlaps everything
    nc.gpsimd.load_library(mlp)

    # ---------------- constants (NEFF Const DRAM -> SBUF, sync queue) -------
    u_strict = const.tile([128, 128], F16, tag="u_strict")
    nc.sync.dma_start(u_strict[:], consts["u_strict"].ap())
    idp = const.tile([128, 128], F16, tag="idp")
    nc.sync.dma_start(idp[:], consts["idp"].ap())
    idn = const.tile([128, 128], F16, tag="idn")
    nc.sync.dma_start(idn[:], consts["idn"].ap())
    u64s = const.tile([NSUB, NSUB], F16, tag="u64s")
    nc.sync.dma_start(u64s[:], consts["u64s"].ap())
    thr_lo = const.tile([NSUB, 1], F32, tag="thr_lo")
    nc.sync.dma_start(thr_lo[:], consts["thr_lo"].ap())
    thr_hi = const.tile([NSUB, 1], F32, tag="thr_hi")
    nc.sync.dma_start(thr_hi[:], consts["thr_hi"].ap())

    # ---------------- span staging ----------------------------------------
    # span-major [p, j]: span n = 128j + p.  These int32 tiles are ALSO the
    # indirect-gather index tables (exclusive table => raw s, e indices).
    st_pj = sbuf.tile([128, NTILE], I32, tag="st_pj")
    en_pj = sbuf.tile([128, NTILE], I32, tag="en_pj")
    mk_pj = sbuf.tile([128, NTILE], I32, tag="mk_pj")
    nc.sync.dma_start(st_pj[:], AP(spans.tensor, 0, [[2, 128], [256, NTILE]]))
    nc.sync.dma_start(en_pj[:], AP(spans.tensor, 1, [[2, 128], [256, NTILE]]))
    nc.sync.dma_start(mk_pj[:], AP(maskw.tensor, 0, [[1, 128], [128, NTILE]]))

    # gather idx list (wrap-16 int16, replicated to 128 partitions):
    # gather t: list pos i in [0,256) = e of span 256t+i; [256,512) = s.
    # dma_gather reads idxs[p, c] = list[c*16 + p%16] -> a32 col 32t + i//16.
    a32 = sbuf.tile([16, 128], I32, tag="a32")
    for t in range(NGATHER):
        nc.sync.dma_start(
            a32[:, 32 * t:32 * t + 16],
            AP(spans.tensor, 512 * t + 1, [[2, 16], [32, 16]]))
        nc.sync.dma_start(
            a32[:, 32 * t + 16:32 * t + 32],
            AP(spans.tensor, 512 * t, [[2, 16], [32, 16]]))
    idx16 = sbuf.tile([128, 128], I16, tag="idx16")
    nc.vector.tensor_copy(idx16[0:16, :], a32[:])
    nc.scalar.dma_start(idx16[16:32, :], idx16[0:16, :])
    nc.scalar.dma_start(idx16[32:64, :], idx16[0:32, :])
    nc.scalar.dma_start(idx16[64:128, :], idx16[0:64, :])

    # prepare gathers now (no RAW dep on the table: traced before stores);
    # each prep costs ~6us of Q7 descgen after the ~25us library load.
    tbl_ap = AP(tbl, 0, [[D, S], [1, D]])
    gsems = [ctx.enter_context(nc.semaphore(f"gsem{t}"))
             for t in range(NGATHER)]
    gts = []
    prep_insts = []
    for t in range(NGATHER):
        g_t = gpool.tile([128, 4, D], F16, tag=f"g{t}")
        pr = nc.gpsimd.dma_gather(
            out_ap=g_t[:], in_ap=tbl_ap, idxs_ap=idx16[:, 32 * t:32 * t + 32],
            num_idxs=512, num_idxs_reg=512, elem_size=D,
            prepare_only=True, sem=gsems[t])
        prep_insts.append(pr)
        gts.append(g_t)

    # free-major onehot rows: cast (e|s) to fp16 (exact, values < 2048), PE
    # transpose [128, 16] -> [16, 128], fold partitions into [1, 1024] rows
    # (span n = 128j + p), replicate to 64 partitions.
    es16 = sbuf.tile([128, 2 * NTILE], F16, tag="es16")
    nc.vector.tensor_copy(es16[:, 0:NTILE], en_pj[:])
    nc.vector.tensor_copy(es16[:, NTILE:2 * NTILE], st_pj[:])
    psT = psum_off.tile([2 * NTILE, 128], F16, tag="psT")
    nc.tensor.transpose(out=psT[:], in_=es16[:], identity=idp[:])
    esT = sbuf.tile([2 * NTILE, 128], F32, tag="esT")
    nc.vector.tensor_copy(esT[:], psT[:])

    ef = sbuf.tile([NSUB, N], F32, tag="ef")
    sf = sbuf.tile([NSUB, N], F32, tag="sf")
    nc.scalar.dma_start(ef[0:1, :].rearrange("p (j c) -> p j c", j=NTILE),
                        esT[0:NTILE, :])
    nc.scalar.dma_start(sf[0:1, :].rearrange("p (j c) -> p j c", j=NTILE),
                        esT[NTILE:2 * NTILE, :])
    for k in (1, 2, 4, 8, 16, 32):
        nc.scalar.dma_start(ef[k:2 * k, :], ef[0:k, :])
        nc.scalar.dma_start(sf[k:2 * k, :], sf[0:k, :])

    # onehot[k, n] = [sub(e_n) == k] - [sub(s_n) == k], via threshold compares
    cme = sbuf.tile([NSUB, N], F32, tag="cme")
    nc.vector.tensor_scalar(out=cme[:], in0=ef[:], scalar1=thr_lo[:],
                            scalar2=None, op0=mybir.AluOpType.is_ge)
    tmp = sbuf.tile([NSUB, N], F32, tag="tmp")
    nc.vector.tensor_scalar(out=tmp[:], in0=ef[:], scalar1=thr_hi[:],
                            scalar2=None, op0=mybir.AluOpType.is_ge)
    nc.vector.tensor_tensor(out=cme[:], in0=cme[:], in1=tmp[:],
                            op=mybir.AluOpType.subtract)
    cms = sbuf.tile([NSUB, N], F32, tag="cms")
    nc.vector.tensor_scalar(out=cms[:], in0=sf[:], scalar1=thr_lo[:],
                            scalar2=None, op0=mybir.AluOpType.is_ge)
    nc.vector.tensor_scalar(out=tmp[:], in0=sf[:], scalar1=thr_hi[:],
                            scalar2=None, op0=mybir.AluOpType.is_ge)
    nc.vector.tensor_tensor(out=cms[:], in0=cms[:], in1=tmp[:],
                            op=mybir.AluOpType.subtract)
    # oh rows 0:64 select C_hi, rows 64:128 the same pattern for C_lo
    oh = sbuf.tile([128, N], F16, tag="oh")
    nc.vector.tensor_tensor(out=oh[0:NSUB, :], in0=cme[:], in1=cms[:],
                            op=mybir.AluOpType.subtract)
    nc.scalar.dma_start(oh[NSUB:128, :], oh[0:NSUB, :])

    # per-span scale = mask / width
    w_i = sbuf.tile([128, NTILE], I32, tag="w_i")
    nc.vector.tensor_tensor(out=w_i[:], in0=en_pj[:], in1=st_pj[:],
                            op=mybir.AluOpType.subtract)
    w_f = sbuf.tile([128, NTILE], F32, tag="w_f")
    nc.vector.tensor_copy(w_f[:], w_i[:])
    r_f = sbuf.tile([128, NTILE], F32, tag="r_f")
    nc.vector.reciprocal(r_f[:], w_f[:])
    m_f = sbuf.tile([128, NTILE], F32, tag="m_f")
    nc.vector.tensor_copy(m_f[:], mk_pj[:])
    scale = sbuf.tile([128, NTILE], F32, tag="scale")
    nc.vector.tensor_tensor(out=scale[:], in0=r_f[:], in1=m_f[:],
                            op=mybir.AluOpType.mult)

    # ---------------- seq load + fp16 cast + local cumsum + table store ----
    store_insts = []
    xbig = sbuf.tile([128, NBLK, D], F32, tag="xbig")
    xf = sbuf.tile([128, NBLK, D], F16, tag="xf")
    tbl_sb = sbuf.tile([128, NBLK, D], F16, tag="tbl_sb")
    for q in range(NBLK // 4):
        sl = (slice(None), slice(4 * q, 4 * q + 4), slice(None))
        nc.sync.dma_start(
            xbig[sl],
            seq[512 * q:512 * (q + 1), :].rearrange("(j p) d -> p j d", p=128))
        nc.vector.tensor_copy(xf[sl], xbig[sl])
        for b in range(4 * q, 4 * q + 4):
            pl = psum_b.tile([128, D], F32, tag="pb")
            nc.tensor.matmul(out=pl[:], lhsT=u_strict[:], rhs=xf[:, b, :],
                             start=True, stop=True)
            if b % 2 == 0:
                nc.scalar.copy(tbl_sb[:, b, :], pl[:])
            else:
                nc.vector.tensor_copy(tbl_sb[:, b, :], pl[:])
        # store 4 blocks (512 rows) to the DRAM table, scalar queue
        store_insts.append(nc.scalar.dma_start(
            AP(tbl, 512 * D * q, [[D, 128], [128 * D, 4], [1, D]]),
            tbl_sb[:, 4 * q:4 * q + 4, :]))

    # ---------------- sub-block totals -> offsets C ------------------------
    # T_b = L[32b + 31] + x[32b + 31]; stored in PERMUTED row order
    # k' = 16i + blk (i = sub-in-block, blk = 128-block); the u64s const's
    # rows are permuted to match, its columns are in true order b.
    t16f = sbuf.tile([NSUB, D], F16, tag="t16f")
    xrow = sbuf.tile([NSUB, D], F16, tag="xrow")
    for i in range(SPB):
        p = SUB * i + SUB - 1
        nc.sync.dma_start(t16f[16 * i:16 * (i + 1), :], tbl_sb[p:p + 1, :, :])
        nc.sync.dma_start(xrow[16 * i:16 * (i + 1), :], xf[p:p + 1, :, :])
    t16 = sbuf.tile([NSUB, D], F32, tag="t16")
    nc.vector.tensor_tensor(out=t16[:], in0=t16f[:], in1=xrow[:],
                            op=mybir.AluOpType.add)
    # hi/lo fp16 split of T, permuted strict-upper matmul -> C = cf (f32)
    th = sbuf.tile([NSUB, D], F16, tag="th")
    nc.vector.tensor_copy(th[:], t16[:])
    tl = sbuf.tile([NSUB, D], F16, tag="tl")
    nc.vector.tensor_tensor(out=tl[:], in0=t16[:], in1=th[:],
                            op=mybir.AluOpType.subtract)
    poff = psum_off.tile([NSUB, D], F32, tag="poff")
    nc.tensor.matmul(out=poff[:], lhsT=u64s[:], rhs=th[:], start=True, stop=False)
    nc.tensor.matmul(out=poff[:], lhsT=u64s[:], rhs=tl[:], start=False, stop=True)
    cf = sbuf.tile([NSUB, D], F32, tag="cf")
    nc.vector.tensor_copy(cf[:], poff[:])
    # hi/lo fp16 split of C, packed [128, 512] for the onehot matmul
    chi = sbuf.tile([NSUB, D], F16, tag="chi")
    nc.vector.tensor_copy(chi[:], cf[:])
    clo = sbuf.tile([NSUB, D], F16, tag="clo")
    nc.vector.tensor_tensor(out=clo[:], in0=cf[:], in1=chi[:],
                            op=mybir.AluOpType.subtract)
    chiclo = sbuf.tile([128, D], F16, tag="chiclo")
    nc.scalar.dma_start(chiclo[0:NSUB, :], chi[:])
    nc.scalar.dma_start(chiclo[NSUB:128, :], clo[:])

    # ---------------- fire the prepared gathers -----------------------------
    for t in range(NGATHER):
        trig = nc.gpsimd.trigger_dma(count=1)
        for st in store_insts:
            add_dep_helper(trig.ins, st.ins, sync=True,
                           reason="gather transfers read table")

    # ---------------- combine: PSUM = onehot@C + tbl[e] - tbl[s] ----------
    for t in range(NGATHER):
        g_t = gts[t]
        for k in range(2):
            j = 2 * t + k
            ps = psum_c.tile([128, D], F32, tag="pc")
            nc.tensor.matmul(out=ps[:], lhsT=oh[:, 128 * j:128 * (j + 1)],
                             rhs=chiclo[:], start=True, stop=False)
            mm_e = nc.tensor.matmul(out=ps[:], lhsT=idp[:], rhs=g_t[:, k, :],
                                    start=False, stop=False)
            mm_e._wait_ge(gsems[t], 16)
            mm_s = nc.tensor.matmul(out=ps[:], lhsT=idn[:], rhs=g_t[:, 2 + k, :],
                                    start=False, stop=True)
            mm_s._wait_ge(gsems[t], 16)
            o_t = opool.tile([128, D], F32, tag="o")
            nc.scalar.mul(o_t[:], ps[:], scale[:, j:j + 1])
            nc.sync.dma_start(out[128 * j:128 * (j + 1), :], o_t[:])

    if dbg is not None:
        nc.sync.dma_start(dbg["tbl_sb"][:], tbl_sb[:])
        nc.sync.dma_start(dbg["ge"][:], ge_all[:])
        nc.sync.dma_start(dbg["gs"][:], gs_all[:])
        nc.sync.dma_start(dbg["oh"][:], oh[:])
        nc.sync.dma_start(dbg["cf"][:], cf[:])
        nc.sync.dma_start(dbg["scale"][:], scale[:])
        nc.sync.dma_start(dbg["t16"][:], t16[:])


def _make_consts(nc):
    # strict-upper within each 32-row sub-block, block-diagonal
    r = np.arange(128)
    ustrict = ((r[:, None] < r[None, :]) &
               (r[:, None] // SUB == r[None, :] // SUB)).astype(np.float16)
    # u64s[k', b] = [true_b(k') < b] with k' = 16i + blk -> true_b = 4*blk + i
    kp = np.arange(NSUB)
    true_b = SPB * (kp % 16) + kp // 16
    u64s = (true_b[:, None] < np.arange(NSUB)[None, :]).astype(np.float16)
    idp = np.eye(128, dtype=np.float16)
    idn = -np.eye(128, dtype=np.float16)
    k = np.arange(NSUB)
    thr_lo = (float(SUB) * k).astype(np.float32).reshape(NSUB, 1)
    thr_hi = (float(SUB) * (k + 1)).astype(np.float32).reshape(NSUB, 1)
    return {
        "u_strict": nc.inline_tensor(ustrict, name="c_ustrict"),
        "u64s": nc.inline_tensor(u64s, name="c_u64s"),
        "idp": nc.inline_tensor(idp, name="c_idp"),
        "idn": nc.inline_tensor(idn, name="c_idn"),
        "thr_lo": nc.inline_tensor(thr_lo, name="c_thrlo"),
        "thr_hi": nc.inline_tensor(thr_hi, name="c_thrhi"),
    }


def build_nc(debug_taps=False):
    nc = bacc.Bacc("TRN2", target_bir_lowering=False, debug=False,
                   dynamic_dma_scratch_size=2 ** 16)
    seq = nc.dram_tensor("seq", [S, D], F32, kind="ExternalInput")
    spans = nc.dram_tensor("spans", [N, 2], I32, kind="ExternalInput")
    maskw = nc.dram_tensor("maskw", [N], I32, kind="ExternalInput")
    out = nc.dram_tensor("out", [N, D], F32, kind="ExternalOutput")
    tbl = nc.dram_tensor("tbl", [S, D], F16, kind="Internal")
    consts = _make_consts(nc)
    dbg = None
    if debug_taps:
        dbg = {
            "tbl_sb": nc.dram_tensor("dbg_tbl", [128, NBLK, D], F16,
                                     kind="ExternalOutput").ap(),
            "ge": nc.dram_tensor("dbg_ge", [128, NTILE, D], F16,
                                 kind="ExternalOutput").ap(),
            "gs": nc.dram_tensor("dbg_gs", [128, NTILE, D], F16,
                                 kind="ExternalOutput").ap(),
            "oh": nc.dram_tensor("dbg_oh", [128, N], F16,
                                 kind="ExternalOutput").ap(),
            "cf": nc.dram_tensor("dbg_cf", [NSUB, D], F32,
                                 kind="ExternalOutput").ap(),
            "scale": nc.dram_tensor("dbg_scale", [128, NTILE], F32,
                                    kind="ExternalOutput").ap(),
            "t16": nc.dram_tensor("dbg_t16", [NSUB, D], F32,
                                  kind="ExternalOutput").ap(),
        }
    from contextlib import ExitStack
    with tile.TileContext(nc) as tc:
        with ExitStack() as ctx:
            build_kernel_body(tc, seq.ap(), spans.ap(), maskw.ap(), out.ap(),
                              tbl, consts, ctx, dbg=dbg)
    nc.compile()
    return nc


_NC_CACHE = None


def kernel(sequence_tensor: np.ndarray, span_indices: np.ndarray,
           span_indices_mask: np.ndarray) -> np.ndarray:
    global _NC_CACHE
    from concourse.bass_utils import run_bass_kernel_spmd

    if _NC_CACHE is None:
        _NC_CACHE = build_nc()
    nc = _NC_CACHE

    spans_i32 = np.ascontiguousarray(np.asarray(span_indices).astype(np.int32))
    mask_i32 = np.ascontiguousarray(np.asarray(span_indices_mask).astype(np.int32))
    seq_f32 = np.ascontiguousarray(sequence_tensor, dtype=np.float32)

    in_maps = [
        {"seq": seq_f32[b], "spans": spans_i32[b], "maskw": mask_i32[b]}
        for b in range(B)
    ]
    res = run_bass_kernel_spmd(nc, in_maps, core_ids=list(range(B)))
    return np.stack([r["out"] for r in res.results], axis=0)


# revision 10
# speedup vs baseline: 5.1403x; 1.0153x over previous
"""AverageSpanExtractor Trainium2 kernel (v3: hardware indirect-DMA gather).

Math: out[b, n, :] = mean(seq[b, s_n:e_n, :]) * mask[b, n]

Per core (data-parallel over batch across 8 cores):
  1. Load seq [S=2048, D=512] f32, cast fp16.
  2. Per 128-token block: block-diag strict-upper matmul gives the 32-row
     SUB-BLOCK-local exclusive prefix L[r] = sum(seq[32*(r>>5) .. r)); cast
     fp16 and store to a DRAM table [2048, 512] fp16.  |L| stays < ~40 so
     fp16 rounding error is ~1e-2 absolute worst-case, well inside the gate.
  3. Gather tbl[e_n] and tbl[s_n] for all spans with indirect hardware DMAs
     (qPoolDynamic; per-partition indices are the RAW span starts/ends since
     the table is exclusive -- no -1 shift, no masking, no gpsimd ucode).
  4. The missing inter-sub-block offsets C[b] = sum of sub-block totals
     [0, b), b in [0, 64), are added exactly via a +-onehot fp16 matmul
     (onehot over e>>5 / s>>5 built from threshold compares) against
     hi/lo-split C vectors.
  5. Per span tile, PSUM accumulates onehot@C + Id@tbl[e] - Id@tbl[s]; one
     ACT pass scales by mask/width and the result is stored.
"""

import numpy as np

import concourse.bacc as bacc
import concourse.bass as bass
import concourse.tile as tile
from concourse import mybir
from concourse.bass import AP
from concourse.library_config import mlp
from concourse.tile_rust import add_dep_helper

# Problem shape (hardcoded per contract).
B, S, D, N = 8, 2048, 512, 1024
NBLK = S // 128          # 16 token blocks (matmul granularity)
SUB = 32                 # sub-block rows (fp16 table locality)
NSUB = S // SUB          # 64 sub-blocks (offset-table entries)
SPB = 128 // SUB         # 4 sub-blocks per 128-block
NTILE = N // 128         # 8 span tiles

F32 = mybir.dt.float32
F16 = mybir.dt.float16
I32 = mybir.dt.int32
I16 = mybir.dt.int16

NGATHER = 4              # gather instructions (2 span tiles each)


def build_kernel_body(tc: tile.TileContext, seq: AP, spans: AP, maskw: AP,
                      out: AP, tbl, consts, ctx, dbg=None):
    nc = tc.nc
    const = ctx.enter_context(tc.tile_pool(name="const", bufs=1))
    sbuf = ctx.enter_context(tc.tile_pool(name="sbuf", bufs=1))
    opool = ctx.enter_context(tc.tile_pool(name="opool", bufs=3))
    gpool = ctx.enter_context(tc.tile_pool(name="gpool", bufs=1))
    psum_b = ctx.enter_context(tc.tile_pool(name="pb", bufs=2, space="PSUM"))
    psum_off = ctx.enter_context(tc.tile_pool(name="poff", bufs=1, space="PSUM"))
    psum_c = ctx.enter_context(tc.tile_pool(name="pc", bufs=4, space="PSUM"))

    # library load first: ~20-25us on the Pool engine, overlaps everything
    nc.gpsimd.load_library(mlp)

    # ---------------- constants (NEFF Const DRAM -> SBUF, sync queue) -------
    u_strict = const.tile([128, 128], F16, tag="u_strict")
    nc.sync.dma_start(u_strict[:], consts["u_strict"].ap())
    idp = const.tile([128, 128], F16, tag="idp")
    nc.sync.dma_start(idp[:], consts["idp"].ap())
    idn = const.tile([128, 128], F16, tag="idn")
    nc.sync.dma_start(idn[:], consts["idn"].ap())
    u64s = const.tile([NSUB, NSUB], F16, tag="u64s")
    nc.sync.dma_start(u64s[:], consts["u64s"].ap())
    thr_lo = const.tile([NSUB, 1], F32, tag="thr_lo")
    nc.sync.dma_start(thr_lo[:], consts["thr_lo"].ap())
    thr_hi = const.tile([NSUB, 1], F32, tag="thr_hi")
    nc.sync.dma_start(thr_hi[:], consts["thr_hi"].ap())

    # ---------------- span staging ----------------------------------------
    # span-major [p, j]: span n = 128j + p.  These int32 tiles are ALSO the
    # indirect-gather index tables (exclusive table => raw s, e indices).
    st_pj = sbuf.tile([128, NTILE], I32, tag="st_pj")
    en_pj = sbuf.tile([128, NTILE], I32, tag="en_pj")
    mk_pj = sbuf.tile([128, NTILE], I32, tag="mk_pj")
    nc.sync.dma_start(st_pj[:], AP(spans.tensor, 0, [[2, 128], [256, NTILE]]))
    nc.sync.dma_start(en_pj[:], AP(spans.tensor, 1, [[2, 128], [256, NTILE]]))
    nc.sync.dma_start(mk_pj[:], AP(maskw.tensor, 0, [[1, 128], [128, NTILE]]))

    # gather idx list (wrap-16 int16, replicated to 128 partitions):
    # gather t: list pos i in [0,256) = e of span 256t+i; [256,512) = s.
    # dma_gather reads idxs[p, c] = list[c*16 + p%16] -> a32 col 32t + i//16.
    a32 = sbuf.tile([16, 128], I32, tag="a32")
    for t in range(NGATHER):
        nc.sync.dma_start(
            a32[:, 32 * t:32 * t + 16],
            AP(spans.tensor, 512 * t + 1, [[2, 16], [32, 16]]))
        nc.sync.dma_start(
            a32[:, 32 * t + 16:32 * t + 32],
            AP(spans.tensor, 512 * t, [[2, 16], [32, 16]]))
    idx16 = sbuf.tile([128, 128], I16, tag="idx16")
    nc.vector.tensor_copy(idx16[0:16, :], a32[:])
    nc.scalar.dma_start(idx16[16:32, :], idx16[0:16, :])
    nc.scalar.dma_start(idx16[32:64, :], idx16[0:32, :])
    nc.scalar.dma_start(idx16[64:128, :], idx16[0:64, :])

    # prepare gathers now (no RAW dep on the table: traced before stores);
    # each prep costs ~6us of Q7 descgen after the ~25us library load.
    tbl_ap = AP(tbl, 0, [[D, S], [1, D]])
    gsems = [ctx.enter_context(nc.semaphore(f"gsem{t}"))
             for t in range(NGATHER)]
    gts = []
    trig_insts = []
    for t in range(NGATHER):
        g_t = gpool.tile([128, 4, D], F16, tag=f"g{t}")
        nc.gpsimd.dma_gather(
            out_ap=g_t[:], in_ap=tbl_ap, idxs_ap=idx16[:, 32 * t:32 * t + 32],
            num_idxs=512, num_idxs_reg=512, elem_size=D,
            prepare_only=True, sem=gsems[t])
        # fire this prep as soon as it lands (table stores finish well before
        # the ~39us library-load+prep point, so the wait is free)
        trig_insts.append(nc.gpsimd.trigger_dma(count=1))
        gts.append(g_t)

    # free-major onehot rows: cast (e|s) to fp16 (exact, values < 2048), PE
    # transpose [128, 16] -> [16, 128], fold partitions into [1, 1024] rows
    # (span n = 128j + p), replicate to 64 partitions.
    es16 = sbuf.tile([128, 2 * NTILE], F16, tag="es16")
    nc.vector.tensor_copy(es16[:, 0:NTILE], en_pj[:])
    nc.vector.tensor_copy(es16[:, NTILE:2 * NTILE], st_pj[:])
    psT = psum_off.tile([2 * NTILE, 128], F16, tag="psT")
    nc.tensor.transpose(out=psT[:], in_=es16[:], identity=idp[:])
    esT = sbuf.tile([2 * NTILE, 128], F32, tag="esT")
    nc.vector.tensor_copy(esT[:], psT[:])

    ef = sbuf.tile([NSUB, N], F32, tag="ef")
    sf = sbuf.tile([NSUB, N], F32, tag="sf")
    nc.scalar.dma_start(ef[0:1, :].rearrange("p (j c) -> p j c", j=NTILE),
                        esT[0:NTILE, :])
    nc.scalar.dma_start(sf[0:1, :].rearrange("p (j c) -> p j c", j=NTILE),
                        esT[NTILE:2 * NTILE, :])
    for k in (1, 2, 4, 8, 16, 32):
        nc.scalar.dma_start(ef[k:2 * k, :], ef[0:k, :])
        nc.scalar.dma_start(sf[k:2 * k, :], sf[0:k, :])

    # onehot[k, n] = [sub(e_n) == k] - [sub(s_n) == k], via threshold compares
    cme = sbuf.tile([NSUB, N], F32, tag="cme")
    nc.vector.tensor_scalar(out=cme[:], in0=ef[:], scalar1=thr_lo[:],
                            scalar2=None, op0=mybir.AluOpType.is_ge)
    tmp = sbuf.tile([NSUB, N], F32, tag="tmp")
    nc.vector.tensor_scalar(out=tmp[:], in0=ef[:], scalar1=thr_hi[:],
                            scalar2=None, op0=mybir.AluOpType.is_ge)
    nc.vector.tensor_tensor(out=cme[:], in0=cme[:], in1=tmp[:],
                            op=mybir.AluOpType.subtract)
    cms = sbuf.tile([NSUB, N], F32, tag="cms")
    nc.vector.tensor_scalar(out=cms[:], in0=sf[:], scalar1=thr_lo[:],
                            scalar2=None, op0=mybir.AluOpType.is_ge)
    nc.vector.tensor_scalar(out=tmp[:], in0=sf[:], scalar1=thr_hi[:],
                            scalar2=None, op0=mybir.AluOpType.is_ge)
    nc.vector.tensor_tensor(out=cms[:], in0=cms[:], in1=tmp[:],
                            op=mybir.AluOpType.subtract)
    # oh rows 0:64 select C_hi, rows 64:128 the same pattern for C_lo
    oh = sbuf.tile([128, N], F16, tag="oh")
    nc.vector.tensor_tensor(out=oh[0:NSUB, :], in0=cme[:], in1=cms[:],
                            op=mybir.AluOpType.subtract)
    nc.scalar.dma_start(oh[NSUB:128, :], oh[0:NSUB, :])

    # per-span scale = mask / width
    w_i = sbuf.tile([128, NTILE], I32, tag="w_i")
    nc.vector.tensor_tensor(out=w_i[:], in0=en_pj[:], in1=st_pj[:],
                            op=mybir.AluOpType.subtract)
    w_f = sbuf.tile([128, NTILE], F32, tag="w_f")
    nc.vector.tensor_copy(w_f[:], w_i[:])
    r_f = sbuf.tile([128, NTILE], F32, tag="r_f")
    nc.vector.reciprocal(r_f[:], w_f[:])
    m_f = sbuf.tile([128, NTILE], F32, tag="m_f")
    nc.vector.tensor_copy(m_f[:], mk_pj[:])
    scale = sbuf.tile([128, NTILE], F32, tag="scale")
    nc.vector.tensor_tensor(out=scale[:], in0=r_f[:], in1=m_f[:],
                            op=mybir.AluOpType.mult)

    # ---------------- seq load + fp16 cast + local cumsum + table store ----
    store_insts = []
    xbig = sbuf.tile([128, NBLK, D], F32, tag="xbig")
    xf = sbuf.tile([128, NBLK, D], F16, tag="xf")
    tbl_sb = sbuf.tile([128, NBLK, D], F16, tag="tbl_sb")
    for q in range(NBLK // 4):
        sl = (slice(None), slice(4 * q, 4 * q + 4), slice(None))
        nc.sync.dma_start(
            xbig[sl],
            seq[512 * q:512 * (q + 1), :].rearrange("(j p) d -> p j d", p=128))
        nc.vector.tensor_copy(xf[sl], xbig[sl])
        for b in range(4 * q, 4 * q + 4):
            pl = psum_b.tile([128, D], F32, tag="pb")
            nc.tensor.matmul(out=pl[:], lhsT=u_strict[:], rhs=xf[:, b, :],
                             start=True, stop=True)
            if b % 2 == 0:
                nc.scalar.copy(tbl_sb[:, b, :], pl[:])
            else:
                nc.vector.tensor_copy(tbl_sb[:, b, :], pl[:])
        # store 4 blocks (512 rows) to the DRAM table, scalar queue
        store_insts.append(nc.scalar.dma_start(
            AP(tbl, 512 * D * q, [[D, 128], [128 * D, 4], [1, D]]),
            tbl_sb[:, 4 * q:4 * q + 4, :]))

    # ---------------- sub-block totals -> offsets C ------------------------
    # T_b = L[32b + 31] + x[32b + 31]; stored in PERMUTED row order
    # k' = 16i + blk (i = sub-in-block, blk = 128-block); the u64s const's
    # rows are permuted to match, its columns are in true order b.
    t16f = sbuf.tile([NSUB, D], F16, tag="t16f")
    xrow = sbuf.tile([NSUB, D], F16, tag="xrow")
    for i in range(SPB):
        p = SUB * i + SUB - 1
        nc.sync.dma_start(t16f[16 * i:16 * (i + 1), :], tbl_sb[p:p + 1, :, :])
        nc.sync.dma_start(xrow[16 * i:16 * (i + 1), :], xf[p:p + 1, :, :])
    t16 = sbuf.tile([NSUB, D], F32, tag="t16")
    nc.vector.tensor_tensor(out=t16[:], in0=t16f[:], in1=xrow[:],
                            op=mybir.AluOpType.add)
    # hi/lo fp16 split of T, permuted strict-upper matmul -> C = cf (f32)
    th = sbuf.tile([NSUB, D], F16, tag="th")
    nc.vector.tensor_copy(th[:], t16[:])
    tl = sbuf.tile([NSUB, D], F16, tag="tl")
    nc.vector.tensor_tensor(out=tl[:], in0=t16[:], in1=th[:],
                            op=mybir.AluOpType.subtract)
    poff = psum_off.tile([NSUB, D], F32, tag="poff")
    nc.tensor.matmul(out=poff[:], lhsT=u64s[:], rhs=th[:], start=True, stop=False)
    nc.tensor.matmul(out=poff[:], lhsT=u64s[:], rhs=tl[:], start=False, stop=True)
    cf = sbuf.tile([NSUB, D], F32, tag="cf")
    nc.vector.tensor_copy(cf[:], poff[:])
    # hi/lo fp16 split of C, packed [128, 512] for the onehot matmul
    chi = sbuf.tile([NSUB, D], F16, tag="chi")
    nc.vector.tensor_copy(chi[:], cf[:])
    clo = sbuf.tile([NSUB, D], F16, tag="clo")
    nc.vector.tensor_tensor(out=clo[:], in0=cf[:], in1=chi[:],
                            op=mybir.AluOpType.subtract)
    chiclo = sbuf.tile([128, D], F16, tag="chiclo")
    nc.scalar.dma_start(chiclo[0:NSUB, :], chi[:])
    nc.scalar.dma_start(chiclo[NSUB:128, :], clo[:])

    # gather triggers were traced before the table stores: add the RAW deps
    for trig in trig_insts:
        for st in store_insts:
            add_dep_helper(trig.ins, st.ins, sync=True,
                           reason="gather transfers read table")

    # ---------------- combine: out = (onehot@C + tbl[e] - tbl[s]) * scale --
    for t in range(NGATHER):
        g_t = gts[t]
        for k in range(2):
            j = 2 * t + k
            ps = psum_c.tile([128, D], F32, tag="pc")
            nc.tensor.matmul(out=ps[:], lhsT=oh[:, 128 * j:128 * (j + 1)],
                             rhs=chiclo[:], start=True, stop=True)
            d_t = opool.tile([128, D], F32, tag="d")
            tt = nc.vector.tensor_tensor(out=d_t[:], in0=g_t[:, k, :],
                                         in1=g_t[:, 2 + k, :],
                                         op=mybir.AluOpType.subtract)
            tt._wait_ge(gsems[t], 16)
            add_dep_helper(tt.ins, trig_insts[t].ins, sync=False,
                           reason="consume after trigger")
            sm = opool.tile([128, D], F32, tag="sm")
            nc.vector.tensor_tensor(out=sm[:], in0=d_t[:], in1=ps[:],
                                    op=mybir.AluOpType.add)
            o_t = opool.tile([128, D], F32, tag="o")
            nc.scalar.mul(o_t[:], sm[:], scale[:, j:j + 1])
            nc.sync.dma_start(out[128 * j:128 * (j + 1), :], o_t[:])

    if dbg is not None:
        nc.sync.dma_start(dbg["tbl_sb"][:], tbl_sb[:])
        nc.sync.dma_start(dbg["ge"][:], ge_all[:])
        nc.sync.dma_start(dbg["gs"][:], gs_all[:])
        nc.sync.dma_start(dbg["oh"][:], oh[:])
        nc.sync.dma_start(dbg["cf"][:], cf[:])
        nc.sync.dma_start(dbg["scale"][:], scale[:])
        nc.sync.dma_start(dbg["t16"][:], t16[:])


def _make_consts(nc):
    # strict-upper within each 32-row sub-block, block-diagonal
    r = np.arange(128)
    ustrict = ((r[:, None] < r[None, :]) &
               (r[:, None] // SUB == r[None, :] // SUB)).astype(np.float16)
    # u64s[k', b] = [true_b(k') < b] with k' = 16i + blk -> true_b = 4*blk + i
    kp = np.arange(NSUB)
    true_b = SPB * (kp % 16) + kp // 16
    u64s = (true_b[:, None] < np.arange(NSUB)[None, :]).astype(np.float16)
    idp = np.eye(128, dtype=np.float16)
    idn = -np.eye(128, dtype=np.float16)
    k = np.arange(NSUB)
    thr_lo = (float(SUB) * k).astype(np.float32).reshape(NSUB, 1)
    thr_hi = (float(SUB) * (k + 1)).astype(np.float32).reshape(NSUB, 1)
    return {
        "u_strict": nc.inline_tensor(ustrict, name="c_ustrict"),
        "u64s": nc.inline_tensor(u64s, name="c_u64s"),
        "idp": nc.inline_tensor(idp, name="c_idp"),
        "idn": nc.inline_tensor(idn, name="c_idn"),
        "thr_lo": nc.inline_tensor(thr_lo, name="c_thrlo"),
        "thr_hi": nc.inline_tensor(thr_hi, name="c_thrhi"),
    }


def build_nc(debug_taps=False):
    nc = bacc.Bacc("TRN2", target_bir_lowering=False, debug=False,
                   dynamic_dma_scratch_size=2 ** 16)
    seq = nc.dram_tensor("seq", [S, D], F32, kind="ExternalInput")
    spans = nc.dram_tensor("spans", [N, 2], I32, kind="ExternalInput")
    maskw = nc.dram_tensor("maskw", [N], I32, kind="ExternalInput")
    out = nc.dram_tensor("out", [N, D], F32, kind="ExternalOutput")
    tbl = nc.dram_tensor("tbl", [S, D], F16, kind="Internal")
    consts = _make_consts(nc)
    dbg = None
    if debug_taps:
        dbg = {
            "tbl_sb": nc.dram_tensor("dbg_tbl", [128, NBLK, D], F16,
                                     kind="ExternalOutput").ap(),
            "ge": nc.dram_tensor("dbg_ge", [128, NTILE, D], F16,
                                 kind="ExternalOutput").ap(),
            "gs": nc.dram_tensor("dbg_gs", [128, NTILE, D], F16,
                                 kind="ExternalOutput").ap(),
            "oh": nc.dram_tensor("dbg_oh", [128, N], F16,
                                 kind="ExternalOutput").ap(),
            "cf": nc.dram_tensor("dbg_cf", [NSUB, D], F32,
                                 kind="ExternalOutput").ap(),
            "scale": nc.dram_tensor("dbg_scale", [128, NTILE], F32,
                                    kind="ExternalOutput").ap(),
            "t16": nc.dram_tensor("dbg_t16", [NSUB, D], F32,
                                  kind="ExternalOutput").ap(),
        }
    from contextlib import ExitStack
    with tile.TileContext(nc) as tc:
        with ExitStack() as ctx:
            build_kernel_body(tc, seq.ap(), spans.ap(), maskw.ap(), out.ap(),
                              tbl, consts, ctx, dbg=dbg)
    nc.compile()
    return nc


_NC_CACHE = None


def kernel(sequence_tensor: np.ndarray, span_indices: np.ndarray,
           span_indices_mask: np.ndarray) -> np.ndarray:
    global _NC_CACHE
    from concourse.bass_utils import run_bass_kernel_spmd

    if _NC_CACHE is None:
        _NC_CACHE = build_nc()
    nc = _NC_CACHE

    spans_i32 = np.ascontiguousarray(np.asarray(span_indices).astype(np.int32))
    mask_i32 = np.ascontiguousarray(np.asarray(span_indices_mask).astype(np.int32))
    seq_f32 = np.ascontiguousarray(sequence_tensor, dtype=np.float32)

    in_maps = [
        {"seq": seq_f32[b], "spans": spans_i32[b], "maskw": mask_i32[b]}
        for b in range(B)
    ]
    res = run_bass_kernel_spmd(nc, in_maps, core_ids=list(range(B)))
    return np.stack([r["out"] for r in res.results], axis=0)
